# revision 12
# baseline (speedup 1.0000x reference)
"""BinaryLinear kernel for Trainium2 (8 NeuronCores, SPMD).

Computes  out = sign(x) @ sign(W)^T * alpha  for
x: [8192, 2048] f32, W: [2048, 2048] f32, alpha: [1] f32.

Strategy: data-parallel over the token dim (8 shards of 1024 tokens);
W replicated. Host side pre-arranges inputs so every DMA the device
issues is a large contiguous read in exact consumption order:
  xt  [KT, 128, TPC]   : x-shard^T, contraction tiles k-major
  wt  [NT, KT, 128, NTS]: W^T in (out-feature slice, k-tile) stream order
  out [NT, MT, 128, NTS]: staged n-major; host re-merges to [TPC, OUTF]
On device: sign() both operands into resident fp8(E4M3) SBUF buffers
(+-1 exact; accumulation of <=2048 +-1 terms is exact in fp32 PSUM),
then DoubleRow fp8 matmuls (2 k-tiles per MM), scale by alpha on
PSUM drain (DVE/ACT alternating), write out per m-pair.

Loop shape: n-outer. Early passes (n0, n1) are k-middle/m-inner so each
arriving k-pair feeds 8 matmuls; resident passes (n2, n3) are
m-outer/k-inner so drains and writes spread out and the tail is short.
"""

import numpy as np

import concourse.bass as bass
import concourse.tile as tile
from concourse import bacc, mybir
from concourse.bass_utils import run_bass_kernel_spmd

N_CORES = 8
NTOK = 8192
INF = 2048
OUTF = 2048
TPC = NTOK // N_CORES  # tokens per core (1024)
P = 128
KT = INF // P  # 16 contraction tiles
MT = TPC // P  # 8 token tiles per core
NTS = 512  # out_features per matmul (one PSUM bank)
NT = OUTF // NTS  # 4

F32 = mybir.dt.float32
FP8 = mybir.dt.float8e4  # E4M3; +-1.0 is exact
SIGN_DT = FP8
K_STEP = 2  # contraction tiles per matmul (2 = fp8 DoubleRow)
KP = KT // K_STEP  # 8 k-pairs

_compiled = None
LAST_RESULT = None  # BassKernelResults of the most recent run (for profiling)


def _build():
    nc = bacc.Bacc(
        "TRN2",
        target_bir_lowering=False,
        debug=False,
        num_devices=N_CORES,
    )
    xt = nc.dram_tensor("xt", [KT, P, TPC], F32, kind="ExternalInput").ap()
    wt = nc.dram_tensor("wt", [NT, KT, P, NTS], F32, kind="ExternalInput").ap()
    al = nc.dram_tensor("alpha", [P, 1], F32, kind="ExternalInput").ap()
    out = nc.dram_tensor("out", [NT, MT, P, NTS], F32, kind="ExternalOutput").ap()

    with tile.TileContext(nc) as tc:
        with (
            tc.tile_pool(name="res", bufs=1) as res,
            tc.tile_pool(name="wload", bufs=4) as wload,
            tc.tile_pool(name="xload", bufs=3) as xload,
            tc.tile_pool(name="psum", bufs=8, space="PSUM") as ppool,
            tc.tile_pool(name="outp", bufs=2) as outp,
        ):
            # Resident sign() buffers (fp8)
            bw = res.tile([P, KT, OUTF], SIGN_DT)  # 32 KB/partition
            bx = res.tile([P, KT, TPC], SIGN_DT)  # 16 KB/partition
            alpha_t = res.tile([P, 1], F32)

            perf_mode = mybir.MatmulPerfMode.DoubleRow if K_STEP == 2 else None

            def mm(ps_ap, m, n, k):
                nc.tensor.matmul(
                    ps_ap,
                    bx[:, k : k + K_STEP, m * P : (m + 1) * P],
                    bw[:, k : k + K_STEP, n * NTS : (n + 1) * NTS],
                    start=(k == 0),
                    stop=(k + K_STEP >= KT),
                    perf_mode=perf_mode,
                )

            def load_sign_w_pair(n, kp):
                # contiguous 512 KB: two k-tiles of one n-slice
                wf = wload.tile([P, K_STEP, NTS], F32, name="wf", tag="wf")
                nc.sync.dma_start(
                    wf[:],
                    wt[n, kp * K_STEP : (kp + 1) * K_STEP].rearrange(
                        "k p c -> p k c"
                    ),
                )
                for j in range(K_STEP):
                    k = kp * K_STEP + j
                    # ACT: sign(w chunk) -> fp8
                    nc.scalar.sign(
                        bw[:, k, n * NTS : (n + 1) * NTS], wf[:, j, :]
                    )

            # ---- load + sign phase (issue order == consumption order) ----
            # x k-pairs (1 MiB, gpsimd ring) + W[kp, n0] (512 KB, sync ring)
            # interleaved, then W n1, n2, n3 pair-chunks on the sync ring.
            for kp in range(KP):
                xf = xload.tile([P, K_STEP, TPC], F32)
                nc.gpsimd.dma_start(
                    xf[:],
                    xt[kp * K_STEP : (kp + 1) * K_STEP].rearrange("k p c -> p k c"),
                )
                for j in range(K_STEP):
                    k = kp * K_STEP + j
                    # DVE: sign(x) as (x > 0) -> {1,0} fp8, then in-place *2-1
                    nc.vector.tensor_scalar(
                        bx[:, k, :], xf[:, j, :], 0.0, None,
                        op0=mybir.AluOpType.is_gt,
                    )
                    nc.vector.tensor_scalar(
                        bx[:, k, :], bx[:, k, :], 2.0, -1.0,
                        op0=mybir.AluOpType.mult, op1=mybir.AluOpType.add,
                    )
                load_sign_w_pair(0, kp)
                if kp == 0:
                    nc.scalar.dma_start(alpha_t[:], al)
            for n in range(1, NT):
                for kp in range(KP):
                    load_sign_w_pair(n, kp)

            def drain(dst, ps, idx):
                # alternate PSUM drains between DVE and ACT
                if idx % 2 == 0:
                    nc.vector.tensor_scalar_mul(dst, ps, alpha_t[:])
                else:
                    nc.scalar.activation(
                        dst, ps, mybir.ActivationFunctionType.Copy,
                        scale=alpha_t[:],
                    )

            # ---- matmul phase ----
            for n in range(NT):
                obuf = outp.tile([P, MT, NTS], F32)
                if n < 2:
                    # streaming passes: k-middle / m-inner
                    pss = [
                        ppool.tile([P, NTS], F32, name="ps", tag="ps")
                        for _ in range(MT)
                    ]
                    for k in range(0, KT, K_STEP):
                        for m in range(MT):
                            mm(pss[m][:], m, n, k)
                    for m in range(MT):
                        drain(obuf[:, m, :], pss[m][:], m)
                        if m % 2 == 1:
                            nc.scalar.dma_start(
                                out[n, m - 1 : m + 1].rearrange("m p c -> p m c"),
                                obuf[:, m - 1 : m + 1, :],
                            )
                else:
                    # resident passes: m-outer / k-inner
                    for m in range(MT):
                        ps = ppool.tile([P, NTS], F32, name="ps", tag="ps")
                        for k in range(0, KT, K_STEP):
                            mm(ps[:], m, n, k)
                        drain(obuf[:, m, :], ps[:], m)
                        if m % 2 == 1:
                            nc.scalar.dma_start(
                                out[n, m - 1 : m + 1].rearrange("m p c -> p m c"),
                                obuf[:, m - 1 : m + 1, :],
                            )

    nc.compile()
    return nc


def kernel(x, weight, alpha):
    global _compiled, LAST_RESULT
    if _compiled is None:
        _compiled = _build()
    nc = _compiled

    x = np.asarray(x, dtype=np.float32)
    weight = np.asarray(weight, dtype=np.float32)
    alpha = np.asarray(alpha, dtype=np.float32)

    # W^T laid out in device stream order: [NT, KT, 128, NTS]
    wt = np.ascontiguousarray(
        weight.T.reshape(KT, P, NT, NTS).transpose(2, 0, 1, 3)
    )
    alv = np.full((P, 1), alpha.reshape(-1)[0], dtype=np.float32)
    in_maps = []
    for c in range(N_CORES):
        xs = np.ascontiguousarray(
            x[c * TPC : (c + 1) * TPC, :].T.reshape(KT, P, TPC)
        )
        in_maps.append({"xt": xs, "wt": wt, "alpha": alv})

    LAST_RESULT = run_bass_kernel_spmd(nc, in_maps, list(range(N_CORES)))
    outs = []
    for c in range(N_CORES):
        o = LAST_RESULT.results[c]["out"]  # [NT, MT, P, NTS]
        outs.append(o.transpose(1, 2, 0, 3).reshape(TPC, OUTF))
    return np.concatenate(outs, axis=0)


# revision 13
# speedup vs baseline: 1.0539x; 1.0539x over previous
"""BinaryLinear kernel for Trainium2 (8 NeuronCores, SPMD).

Computes  out = sign(x) @ sign(W)^T * alpha  for
x: [8192, 2048] f32, W: [2048, 2048] f32, alpha: [1] f32.

Strategy: data-parallel over the token dim (8 shards of 1024 tokens);
W replicated. Host side packs inputs into flat per-chunk streams so
every DMA is a single fully-contiguous transfer with 4-8 KB runs per
SBUF partition, in exact consumption order. On device: sign() both
operands into resident fp8(E4M3) SBUF buffers (+-1 exact; accumulation
of <=2048 +-1 terms is exact in fp32 PSUM), then DoubleRow fp8 matmuls
(2 k-tiles per MM), scale by alpha on PSUM drain (DVE/ACT
alternating), write out per m-pair (contiguous staging layout, host
re-merges).

Rings: sync carries W slices n0 (pair chunks) n1, n2 (quad chunks);
gpsimd carries x (pair chunks) then W n3; scalar carries alpha and
output writes.
"""

import numpy as np

import concourse.bass as bass
import concourse.tile as tile
from concourse import bacc, mybir
from concourse.bass_utils import run_bass_kernel_spmd

N_CORES = 8
NTOK = 8192
INF = 2048
OUTF = 2048
TPC = NTOK // N_CORES  # tokens per core (1024)
P = 128
KT = INF // P  # 16 contraction tiles
MT = TPC // P  # 8 token tiles per core
NTS = 512  # out_features per matmul (one PSUM bank)
NT = OUTF // NTS  # 4

F32 = mybir.dt.float32
FP8 = mybir.dt.float8e4  # E4M3; +-1.0 is exact
SIGN_DT = FP8
K_STEP = 2  # contraction tiles per matmul (2 = fp8 DoubleRow)

# W chunk schedule per n-slice: n0 in k-pairs (fine-grained pacing while
# x streams), n1..n3 in k-quads (1 MiB chunks, 8 KB/partition runs).
W_CHUNKS = {0: [2] * 8, 1: [4] * 4, 2: [4] * 4, 3: [4] * 4}
X_CHUNKS = [2] * 8

_compiled = None
LAST_RESULT = None  # BassKernelResults of the most recent run (for profiling)


def _build():
    nc = bacc.Bacc(
        "TRN2",
        target_bir_lowering=False,
        debug=False,
        num_devices=N_CORES,
    )
    xt = nc.dram_tensor("xt", [KT * P * TPC], F32, kind="ExternalInput").ap()
    wt = nc.dram_tensor("wt", [NT * KT * P * NTS], F32, kind="ExternalInput").ap()
    al = nc.dram_tensor("alpha", [P, 1], F32, kind="ExternalInput").ap()
    out = nc.dram_tensor(
        "out", [NT, MT // 2, P, 2 * NTS], F32, kind="ExternalOutput"
    ).ap()

    with tile.TileContext(nc) as tc:
        with (
            tc.tile_pool(name="res", bufs=1) as res,
            tc.tile_pool(name="wload", bufs=4) as wload,
            tc.tile_pool(name="xload", bufs=3) as xload,
            tc.tile_pool(name="psum", bufs=8, space="PSUM") as ppool,
            tc.tile_pool(name="outp", bufs=2) as outp,
        ):
            # Resident sign() buffers (fp8)
            bw = res.tile([P, KT, OUTF], SIGN_DT)  # 32 KB/partition
            bx = res.tile([P, KT, TPC], SIGN_DT)  # 16 KB/partition
            alpha_t = res.tile([P, 1], F32)

            perf_mode = mybir.MatmulPerfMode.DoubleRow if K_STEP == 2 else None

            def mm(ps_ap, m, n, k):
                nc.tensor.matmul(
                    ps_ap,
                    bx[:, k : k + K_STEP, m * P : (m + 1) * P],
                    bw[:, k : k + K_STEP, n * NTS : (n + 1) * NTS],
                    start=(k == 0),
                    stop=(k + K_STEP >= KT),
                    perf_mode=perf_mode,
                )

            w_off = [0]

            def load_sign_w_chunk(n, k0, sz, engine):
                wf = wload.tile([P, sz, NTS], F32, name="wf", tag="wf")
                src = wt[w_off[0] : w_off[0] + P * sz * NTS].rearrange(
                    "(p f) -> p f", p=P
                )
                engine.dma_start(wf[:].rearrange("p a b -> p (a b)"), src)
                w_off[0] += P * sz * NTS
                for j in range(sz):
                    nc.scalar.sign(bw[:, k0 + j, n * NTS : (n + 1) * NTS], wf[:, j, :])

            x_off = [0]

            def load_sign_x_chunk(k0, sz):
                xf = xload.tile([P, sz, TPC], F32, name="xf", tag="xf")
                src = xt[x_off[0] : x_off[0] + P * sz * TPC].rearrange(
                    "(p f) -> p f", p=P
                )
                nc.gpsimd.dma_start(xf[:].rearrange("p a b -> p (a b)"), src)
                x_off[0] += P * sz * TPC
                for j in range(sz):
                    nc.vector.tensor_scalar(
                        bx[:, k0 + j, :], xf[:, j, :], 0.0, None,
                        op0=mybir.AluOpType.is_gt,
                    )
                    nc.vector.tensor_scalar(
                        bx[:, k0 + j, :], bx[:, k0 + j, :], 2.0, -1.0,
                        op0=mybir.AluOpType.mult, op1=mybir.AluOpType.add,
                    )

            # ---- load + sign phase (issue order == consumption order) ----
            # sync ring: W n0 pairs (interleaved with x), then n1, n2 quads.
            # gpsimd ring: x pairs, then W n3 quads.
            k0 = 0
            for i, sz in enumerate(X_CHUNKS):
                load_sign_x_chunk(k0, sz)
                load_sign_w_chunk(0, k0, W_CHUNKS[0][i], nc.sync)
                k0 += sz
                if i == 0:
                    nc.scalar.dma_start(alpha_t[:], al)
            for n in (1, 2):
                k0 = 0
                for sz in W_CHUNKS[n]:
                    load_sign_w_chunk(n, k0, sz, nc.sync)
                    k0 += sz
            k0 = 0
            for sz in W_CHUNKS[3]:
                load_sign_w_chunk(3, k0, sz, nc.gpsimd)
                k0 += sz

            def drain(dst, ps, idx):
                # alternate PSUM drains between DVE and ACT
                if idx % 2 == 0:
                    nc.vector.tensor_scalar_mul(dst, ps, alpha_t[:])
                else:
                    nc.scalar.activation(
                        dst, ps, mybir.ActivationFunctionType.Copy,
                        scale=alpha_t[:],
                    )

            # ---- matmul phase ----
            for n in range(NT):
                obuf = outp.tile([P, MT, NTS], F32)
                if n < 2:
                    # streaming passes: k-middle / m-inner
                    pss = [
                        ppool.tile([P, NTS], F32, name="ps", tag="ps")
                        for _ in range(MT)
                    ]
                    for k in range(0, KT, K_STEP):
                        for m in range(MT):
                            mm(pss[m][:], m, n, k)
                    for m in range(MT):
                        drain(obuf[:, m, :], pss[m][:], m)
                        if m % 2 == 1:
                            nc.scalar.dma_start(
                                out[n, m // 2],
                                obuf[:, m - 1 : m + 1, :].rearrange(
                                    "p a b -> p (a b)"
                                ),
                            )
                else:
                    # resident passes: m-outer / k-inner
                    for m in range(MT):
                        ps = ppool.tile([P, NTS], F32, name="ps", tag="ps")
                        for k in range(0, KT, K_STEP):
                            mm(ps[:], m, n, k)
                        drain(obuf[:, m, :], ps[:], m)
                        if m % 2 == 1:
                            nc.scalar.dma_start(
                                out[n, m // 2],
                                obuf[:, m - 1 : m + 1, :].rearrange(
                                    "p a b -> p (a b)"
                                ),
                            )

    nc.compile()
    return nc


def _pack_w(weight):
    # WT4[k, p, n, c] = W^T[(k*128+p), n*512+c]
    wt4 = weight.T.reshape(KT, P, NT, NTS)
    parts = []
    for n in range(NT):
        k0 = 0
        for sz in W_CHUNKS[n]:
            parts.append(
                wt4[k0 : k0 + sz, :, n, :].transpose(1, 0, 2).ravel()
            )
            k0 += sz
    return np.ascontiguousarray(np.concatenate(parts))


def _pack_x_shard(xs):
    # xs: [TPC, INF] -> xT4[k, p, t]
    xt4 = xs.T.reshape(KT, P, TPC)
    parts = []
    k0 = 0
    for sz in X_CHUNKS:
        parts.append(xt4[k0 : k0 + sz].transpose(1, 0, 2).ravel())
        k0 += sz
    return np.ascontiguousarray(np.concatenate(parts))


def kernel(x, weight, alpha):
    global _compiled, LAST_RESULT
    if _compiled is None:
        _compiled = _build()
    nc = _compiled

    x = np.asarray(x, dtype=np.float32)
    weight = np.asarray(weight, dtype=np.float32)
    alpha = np.asarray(alpha, dtype=np.float32)

    wt = _pack_w(weight)
    alv = np.full((P, 1), alpha.reshape(-1)[0], dtype=np.float32)
    in_maps = []
    for c in range(N_CORES):
        xs = _pack_x_shard(x[c * TPC : (c + 1) * TPC, :])
        in_maps.append({"xt": xs, "wt": wt, "alpha": alv})

    LAST_RESULT = run_bass_kernel_spmd(nc, in_maps, list(range(N_CORES)))
    outs = []
    for c in range(N_CORES):
        o = LAST_RESULT.results[c]["out"]  # [NT, MT//2, P, 2*NTS]
        o = o.reshape(NT, MT // 2, P, 2, NTS)
        # -> [MT//2, 2, P, NT, NTS] -> [TPC, OUTF]
        outs.append(o.transpose(1, 3, 2, 0, 4).reshape(TPC, OUTF))
    return np.concatenate(outs, axis=0)


# revision 17
# speedup vs baseline: 1.0998x; 1.0436x over previous
"""BinaryLinear kernel for Trainium2 (8 NeuronCores, SPMD).

Computes  out = sign(x) @ sign(W)^T * alpha  for
x: [8192, 2048] f32, W: [2048, 2048] f32, alpha: [1] f32.

Strategy: data-parallel over the token dim (8 shards of 1024 tokens);
W replicated. Host side packs inputs into flat per-chunk streams so
every DMA is a single fully-contiguous transfer with 4-8 KB runs per
SBUF partition, in exact consumption order. On device: sign() both
operands into resident fp8(E4M3) SBUF buffers (+-1 exact; accumulation
of <=2048 +-1 terms is exact in fp32 PSUM), then DoubleRow fp8 matmuls
(2 k-tiles per MM), scale by alpha on PSUM drain (DVE/ACT
alternating), write out per m-pair (contiguous staging layout, host
re-merges).

Rings: sync carries W slices n0 (pair chunks) n1, n2 (quad chunks);
gpsimd carries x (pair chunks) then W n3; scalar carries alpha and
output writes.
"""

import numpy as np

import concourse.bass as bass
import concourse.tile as tile
from concourse import bacc, mybir
from concourse.bass_utils import run_bass_kernel_spmd

N_CORES = 8
NTOK = 8192
INF = 2048
OUTF = 2048
TPC = NTOK // N_CORES  # tokens per core (1024)
P = 128
KT = INF // P  # 16 contraction tiles
MT = TPC // P  # 8 token tiles per core
NTS = 512  # out_features per matmul (one PSUM bank)
NT = OUTF // NTS  # 4

F32 = mybir.dt.float32
FP8 = mybir.dt.float8e4  # E4M3; +-1.0 is exact
SIGN_DT = FP8
K_STEP = 2  # contraction tiles per matmul (2 = fp8 DoubleRow)

# W chunk schedule per n-slice: n0 in small chunks (fine-grained pacing
# while x streams, tiny first chunks to fill the pipeline), n1..n3 in
# k-quads (1 MiB chunks, 8 KB/partition runs).
W_CHUNKS = {0: [1, 1, 2, 2, 2, 4, 4], 1: [4] * 4, 2: [4] * 4, 3: [4] * 4}
X_CHUNKS = [1, 1, 2, 4, 4, 4]

_compiled = None
LAST_RESULT = None  # BassKernelResults of the most recent run (for profiling)


def _build():
    nc = bacc.Bacc(
        "TRN2",
        target_bir_lowering=False,
        debug=False,
        num_devices=N_CORES,
    )
    xt = nc.dram_tensor("xt", [KT * P * TPC], F32, kind="ExternalInput").ap()
    wt = nc.dram_tensor("wt", [NT * KT * P * NTS], F32, kind="ExternalInput").ap()
    al = nc.dram_tensor("alpha", [P, 1], F32, kind="ExternalInput").ap()
    out = nc.dram_tensor(
        "out", [NT, MT // 2, P, 2 * NTS], F32, kind="ExternalOutput"
    ).ap()

    with tile.TileContext(nc) as tc:
        with (
            tc.tile_pool(name="res", bufs=1) as res,
            tc.tile_pool(name="wload", bufs=4) as wload,
            tc.tile_pool(name="xload", bufs=3) as xload,
            tc.tile_pool(name="psum", bufs=8, space="PSUM") as ppool,
            tc.tile_pool(name="outp", bufs=2) as outp,
        ):
            # Resident sign() buffers (fp8)
            bw = res.tile([P, KT, OUTF], SIGN_DT)  # 32 KB/partition
            bx = res.tile([P, KT, TPC], SIGN_DT)  # 16 KB/partition
            alpha_t = res.tile([P, 1], F32)

            perf_mode = mybir.MatmulPerfMode.DoubleRow if K_STEP == 2 else None

            def mm(ps_ap, m, n, k):
                nc.tensor.matmul(
                    ps_ap,
                    bx[:, k : k + K_STEP, m * P : (m + 1) * P],
                    bw[:, k : k + K_STEP, n * NTS : (n + 1) * NTS],
                    start=(k == 0),
                    stop=(k + K_STEP >= KT),
                    perf_mode=perf_mode,
                )

            w_off = [0]

            def load_sign_w_chunk(n, k0, sz, engine):
                wf = wload.tile([P, sz, NTS], F32, name="wf", tag="wf")
                src = wt[w_off[0] : w_off[0] + P * sz * NTS].rearrange(
                    "(p f) -> p f", p=P
                )
                engine.dma_start(wf[:].rearrange("p a b -> p (a b)"), src)
                w_off[0] += P * sz * NTS
                for j in range(sz):
                    nc.scalar.sign(bw[:, k0 + j, n * NTS : (n + 1) * NTS], wf[:, j, :])

            x_off = [0]

            def load_sign_x_chunk(k0, sz):
                xf = xload.tile([P, sz, TPC], F32, name="xf", tag="xf")
                src = xt[x_off[0] : x_off[0] + P * sz * TPC].rearrange(
                    "(p f) -> p f", p=P
                )
                nc.gpsimd.dma_start(xf[:].rearrange("p a b -> p (a b)"), src)
                x_off[0] += P * sz * TPC
                for j in range(sz):
                    nc.vector.tensor_scalar(
                        bx[:, k0 + j, :], xf[:, j, :], 0.0, None,
                        op0=mybir.AluOpType.is_gt,
                    )
                    nc.vector.tensor_scalar(
                        bx[:, k0 + j, :], bx[:, k0 + j, :], 2.0, -1.0,
                        op0=mybir.AluOpType.mult, op1=mybir.AluOpType.add,
                    )

            # ---- load + sign phase (issue order == consumption order) ----
            # gpsimd ring: x chunks. sync ring: all W chunks, n0 first
            # (interleaved with x by k-progress), then n1, n2, n3.
            xi = wi = xk = wk = 0
            first = True
            while xi < len(X_CHUNKS) or wi < len(W_CHUNKS[0]):
                if xi < len(X_CHUNKS) and (wi >= len(W_CHUNKS[0]) or xk <= wk):
                    load_sign_x_chunk(xk, X_CHUNKS[xi])
                    xk += X_CHUNKS[xi]
                    xi += 1
                else:
                    load_sign_w_chunk(0, wk, W_CHUNKS[0][wi], nc.sync)
                    wk += W_CHUNKS[0][wi]
                    wi += 1
                if first:
                    nc.scalar.dma_start(alpha_t[:], al)
                    first = False
            for n in (1, 2, 3):
                k0 = 0
                for sz in W_CHUNKS[n]:
                    load_sign_w_chunk(n, k0, sz, nc.sync)
                    k0 += sz

            def drain(dst, ps, idx, last_pass):
                # DVE drains mid-kernel (ACT is busy signing); alternate
                # DVE/ACT in the last pass so the tail drains in parallel.
                if not last_pass or idx % 2 == 0:
                    nc.vector.tensor_scalar_mul(dst, ps, alpha_t[:])
                else:
                    nc.scalar.activation(
                        dst, ps, mybir.ActivationFunctionType.Copy,
                        scale=alpha_t[:],
                    )

            # ---- matmul phase ----
            for n in range(NT):
                obuf = outp.tile([P, MT, NTS], F32)
                if n < 2:
                    # streaming passes: k-middle / m-inner
                    pss = [
                        ppool.tile([P, NTS], F32, name="ps", tag="ps")
                        for _ in range(MT)
                    ]
                    for k in range(0, KT, K_STEP):
                        for m in range(MT):
                            mm(pss[m][:], m, n, k)
                    for m in range(MT):
                        drain(obuf[:, m, :], pss[m][:], m, n == NT - 1)
                        if m % 2 == 1:
                            nc.scalar.dma_start(
                                out[n, m // 2],
                                obuf[:, m - 1 : m + 1, :].rearrange(
                                    "p a b -> p (a b)"
                                ),
                            )
                else:
                    # resident passes: m-outer / k-inner
                    for m in range(MT):
                        ps = ppool.tile([P, NTS], F32, name="ps", tag="ps")
                        for k in range(0, KT, K_STEP):
                            mm(ps[:], m, n, k)
                        drain(obuf[:, m, :], ps[:], m, n == NT - 1)
                        if m % 2 == 1:
                            nc.scalar.dma_start(
                                out[n, m // 2],
                                obuf[:, m - 1 : m + 1, :].rearrange(
                                    "p a b -> p (a b)"
                                ),
                            )

    nc.compile()
    return nc


def _pack_w(weight):
    # WT4[k, p, n, c] = W^T[(k*128+p), n*512+c]
    wt4 = weight.T.reshape(KT, P, NT, NTS)
    parts = []
    for n in range(NT):
        k0 = 0
        for sz in W_CHUNKS[n]:
            parts.append(
                wt4[k0 : k0 + sz, :, n, :].transpose(1, 0, 2).ravel()
            )
            k0 += sz
    return np.ascontiguousarray(np.concatenate(parts))


def _pack_x_shard(xs):
    # xs: [TPC, INF] -> xT4[k, p, t]
    xt4 = xs.T.reshape(KT, P, TPC)
    parts = []
    k0 = 0
    for sz in X_CHUNKS:
        parts.append(xt4[k0 : k0 + sz].transpose(1, 0, 2).ravel())
        k0 += sz
    return np.ascontiguousarray(np.concatenate(parts))


def kernel(x, weight, alpha):
    global _compiled, LAST_RESULT
    if _compiled is None:
        _compiled = _build()
    nc = _compiled

    x = np.asarray(x, dtype=np.float32)
    weight = np.asarray(weight, dtype=np.float32)
    alpha = np.asarray(alpha, dtype=np.float32)

    wt = _pack_w(weight)
    alv = np.full((P, 1), alpha.reshape(-1)[0], dtype=np.float32)
    in_maps = []
    for c in range(N_CORES):
        xs = _pack_x_shard(x[c * TPC : (c + 1) * TPC, :])
        in_maps.append({"xt": xs, "wt": wt, "alpha": alv})

    LAST_RESULT = run_bass_kernel_spmd(nc, in_maps, list(range(N_CORES)))
    outs = []
    for c in range(N_CORES):
        o = LAST_RESULT.results[c]["out"]  # [NT, MT//2, P, 2*NTS]
        o = o.reshape(NT, MT // 2, P, 2, NTS)
        # -> [MT//2, 2, P, NT, NTS] -> [TPC, OUTF]
        outs.append(o.transpose(1, 3, 2, 0, 4).reshape(TPC, OUTF))
    return np.concatenate(outs, axis=0)
